# revision 6
# baseline (speedup 1.0000x reference)
"""GAT kernel v2 for Trainium2, 8-core SPMD.

Changes vs baseline:
  - Snake (boustrophedon) node binning: gather padding 1.52x -> 1.35x.
  - Optional per-core trailing truncation of gather calls via reg_load'd
    num_idxs_reg (TRUNC): effective padding -> per-core ~1.23x.
  - Stage A: host-pretransposed X (no per-tile PE transpose); e_src/e_dst
    computed by PE in the same matmul as h (fused [wt | wa] rhs, wa built
    on device from host-permuted W/a layouts); he rows written 272B.
  - Stage C: exp on ACT engine with per-partition bias=-max and accum_out
    z (removes two DVE passes); reduce_max emits negated max directly.
"""
import sys

if "/opt/trn_rl_repo" not in sys.path:
    sys.path.insert(0, "/opt/trn_rl_repo")

import hashlib
import numpy as np

N, DEG, K, F_IN, F_OUT = 100000, 16, 4, 128, 32
KF = K * F_OUT            # 128
N_CORES = 8
S = N // N_CORES          # 12500
P = 128
NT = (S + P - 1) // P     # 98
SP = NT * P               # 12544
NTAB = N_CORES * SP       # 100352
EL = 256                  # bf16 elements per table row (512B)
EOFF = 128                # h at [0,128); e_dst hi bf16 [128,132); lo [132,136)
HEW = 136                 # written row width (h + e hi/lo)
BOUNDS = (0, 2 * SP, 4 * SP, 6 * SP, NTAB)   # shard-pair aligned (25088)
NW = 4
DUMMY = (S, 2 * SP + S, 4 * SP + S, 6 * SP + S)
NEG_SLOPE = 0.01
NEG_BIG = -1.0e30
TRUNC = False
SINGLE_PACKET = False


def build_nc(J, n_cores=N_CORES, nt=NT, trunc=TRUNC, ps0=S):
    """J: [nt, NW] int array of per-tile window slot counts (uniform across
    cores). ps0: first pad row in the (tile-permuted) device row order.
    Builds and compiles the SPMD program."""
    from contextlib import ExitStack

    import concourse.bass as bass
    import concourse.tile as tile
    from concourse import bacc, mybir

    f32 = mybir.dt.float32
    bf16 = mybir.dt.bfloat16
    i16 = mybir.dt.int16
    i32 = mybir.dt.int32
    sp = nt * P
    Jt = J.sum(axis=1)            # slots per tile
    JTM = int(Jt.max())
    CTOT = int(J.sum()) * 8       # idxbuf columns (16-wrapped)
    NCALL = int((J > 0).sum())

    nc = bacc.Bacc("TRN2", target_bir_lowering=False, debug=False,
                   num_devices=n_cores, num_swdge_queues=4)

    CTJ = int(Jt.sum())
    xst = nc.dram_tensor("xst", [F_IN, sp], f32, kind="ExternalInput")
    wt = nc.dram_tensor("wt", [F_IN, KF], f32, kind="ExternalInput")
    wkf = nc.dram_tensor("wkf", [KF, F_IN], f32, kind="ExternalInput")
    am = nc.dram_tensor("am", [KF, 8], f32, kind="ExternalInput")
    idxin = nc.dram_tensor("idxin", [P, CTOT], i16, kind="ExternalInput")
    nidxin = nc.dram_tensor("nidxin", [1, NCALL], i32, kind="ExternalInput")
    mskin = nc.dram_tensor("mskin", [1, CTJ], f32, kind="ExternalInput")
    padfill = nc.dram_tensor("padfill", [sp - S if sp > S else 1, 8], bf16,
                             kind="ExternalInput")
    out = nc.dram_tensor("out", [sp, KF], f32, kind="ExternalOutput")

    he_shard = nc.dram_tensor("he_shard", [sp, EL], bf16, kind="Internal")
    he_full = nc.dram_tensor("he_full", [NTAB, EL], bf16, kind="Internal",
                             addr_space="Shared")

    with tile.TileContext(nc) as tc, ExitStack() as ctx:
        consts = ctx.enter_context(tc.tile_pool(name="consts", bufs=1))
        sa = ctx.enter_context(tc.tile_pool(name="sa", bufs=4))
        sa_ps = ctx.enter_context(tc.tile_pool(name="sa_ps", bufs=4, space="PSUM"))
        sc = ctx.enter_context(tc.tile_pool(name="sc", bufs=4))
        scg = ctx.enter_context(tc.tile_pool(name="scg", bufs=5))
        swg = ctx.enter_context(tc.tile_pool(name="swg", bufs=4))

        wkf_sb = consts.tile([KF, F_IN], f32)
        nc.sync.dma_start(wkf_sb[:], wkf.ap())
        am_sb = consts.tile([KF, 8], f32)
        nc.sync.dma_start(am_sb[:], am.ap())
        rhs_sb = consts.tile([F_IN, KF + 8], f32)
        nc.sync.dma_start(rhs_sb[:, 0:KF], wt.ap())
        wa_ps = sa_ps.tile([F_IN, 8], f32, tag="wa")
        nc.tensor.matmul(wa_ps[:], lhsT=wkf_sb[:], rhs=am_sb[:],
                         start=True, stop=True)
        nc.vector.tensor_copy(rhs_sb[:, KF:KF + 8], wa_ps[:])

        es_sb = consts.tile([P, nt * K], f32)
        idx_sb = consts.tile([P, CTOT], i16)
        nc.sync.dma_start(idx_sb[:], idxin.ap())
        nidx_sb = consts.tile([1, NCALL], i32)
        nc.sync.dma_start(nidx_sb[:], nidxin.ap())
        msk_sb = None
        if trunc:
            # per-slot mask (0 valid / -1e30 truncated), replicated to all
            # partitions by a partition-stride-0 DMA read
            msk_sb = consts.tile([P, CTJ], f32)
            nc.sync.dma_start(msk_sb[:], bass.AP(mskin, 0, [[0, P], [1, CTJ]]))

        # ---- Stage A ----
        # 4 node-tiles per DMA instruction (in and out) to cut the SP
        # queue's per-DMA dispatch/sem chain 4x
        GA = 4
        for t0 in range(0, nt, GA):
            gg = min(GA, nt - t0)
            xt4 = sa.tile([F_IN, GA * P], f32, tag="x")
            nc.sync.dma_start(xt4[:, 0:gg * P],
                              xst.ap()[:, t0 * P:(t0 + gg) * P])
            he4 = sa.tile([P, GA * HEW], bf16, tag="he")
            for g in range(gg):
                t = t0 + g
                xt_sb = xt4[:, g * P:(g + 1) * P]
                he8_ps = sa_ps.tile([P, KF + 8], f32, tag="he8")
                nc.tensor.matmul(he8_ps[:], lhsT=xt_sb, rhs=rhs_sb[:],
                                 start=True, stop=True)
                he_t = he4[:, g * HEW:(g + 1) * HEW]
                nc.scalar.copy(he_t[:, 0:KF], he8_ps[:, 0:KF])  # f32 -> bf16
                nc.vector.tensor_copy(es_sb[:, t * K:(t + 1) * K],
                                      he8_ps[:, KF:KF + 4])
                # e_dst as hi+lo bf16 pair (~16-bit mantissa total)
                nc.vector.tensor_copy(he_t[:, EOFF:EOFF + 4],
                                      he8_ps[:, KF + 4:KF + 8])
                ehi32 = sa.tile([P, K], f32, tag="ehi32")
                nc.vector.tensor_copy(ehi32[:], he_t[:, EOFF:EOFF + 4])
                elo = sa.tile([P, K], f32, tag="elo")
                nc.vector.tensor_sub(elo[:], he8_ps[:, KF + 4:KF + 8],
                                     ehi32[:])
                nc.vector.tensor_copy(he_t[:, EOFF + 4:EOFF + 8], elo[:])
            dst = bass.AP(he_shard, t0 * P * EL,
                          [[EL, P], [P * EL, gg], [1, HEW]])
            nc.sync.dma_start(
                dst, he4[:, 0:gg * HEW].rearrange("p (g c) -> p g c", c=HEW))
        npad = sp - S
        if npad > 0:
            pf = consts.tile([npad, 8], bf16)
            nc.sync.dma_start(pf[:], padfill.ap())
            nc.sync.dma_start(
                he_shard.ap()[ps0:ps0 + npad, EOFF:EOFF + 8], pf[:])

        # ---- Stage B ----
        nc.gpsimd.collective_compute(
            "AllGather", mybir.AluOpType.bypass,
            replica_groups=[list(range(n_cores))],
            ins=[he_shard.ap()], outs=[he_full.ap()])

        # ---- Stage C ----
        # Tiles are host-ordered so the 3 largest-jt tiles run first and
        # untruncated: they fully initialize the 3 rotating gather buffers,
        # so later (truncated) tiles only ever read slots some predecessor
        # wrote (no NaN from uninitialized SBUF, no warm-up memsets needed).
        regs = [ctx.enter_context(nc.gpsimd.register(f"nreg{i}"))
                for i in range(4)]

        coff = 0
        moff = 0
        ncall = 0
        for t in range(nt):
            jt = int(Jt[t])
            g = scg.tile([P, JTM * EL], bf16, tag="g")
            g3 = g[:, 0:jt * EL].rearrange("p (d w) -> p d w", w=EL)
            b = 0
            for w in range(NW):
                jw = int(J[t, w])
                if jw == 0:
                    continue
                nidx = jw * P
                if trunc:
                    reg = regs[ncall % 4]
                    nc.gpsimd.reg_load(reg, nidx_sb[0:1, ncall:ncall + 1])
                    nreg = reg
                else:
                    nreg = nidx
                nc.gpsimd.dma_gather(
                    out_ap=g3[:, b:b + jw, :],
                    in_ap=he_full.ap()[BOUNDS[w]:BOUNDS[w + 1], :],
                    idxs_ap=idx_sb[:, coff:coff + jw * 8],
                    num_idxs=nidx, num_idxs_reg=nreg, elem_size=EL,
                    single_packet=SINGLE_PACKET, queue_num=(w + t) % 4)
                ncall += 1
                b += jw
                coff += jw * 8
            # scores (k-major over slots): e_dst = hi + lo, then + e_src
            ehi = g3[:, :, EOFF:EOFF + 4].rearrange("p d k -> p k d")
            elo_v = g3[:, :, EOFF + 4:EOFF + 8].rearrange("p d k -> p k d")
            ed = sc.tile([P, K * jt], f32, tag="ed")
            edv = ed[:].rearrange("p (k d) -> p k d", d=jt)
            nc.vector.tensor_add(edv, ehi, elo_v)
            esrc = es_sb[:, t * K:(t + 1) * K].unsqueeze(-1).to_broadcast(
                [P, K, jt])
            s0 = sc.tile([P, K * jt], f32, tag="s0")
            s0v = s0[:].rearrange("p (k d) -> p k d", d=jt)
            nc.vector.tensor_add(s0v, edv, esrc)
            if trunc:
                mskv = msk_sb[:, moff:moff + jt].unsqueeze(1).to_broadcast(
                    [P, K, jt])
                nc.vector.tensor_add(s0v, s0v, mskv)
                moff += jt
            s1 = sc.tile([P, K * jt], f32, tag="s1")
            nc.vector.scalar_tensor_tensor(
                s1[:], s0[:], NEG_SLOPE, s0[:],
                op0=mybir.AluOpType.mult, op1=mybir.AluOpType.max)
            s1v = s1[:].rearrange("p (k d) -> p k d", d=jt)
            negm = sc.tile([P, K], f32, tag="negm")
            nc.vector.reduce_max(negm[:], s1v, axis=mybir.AxisListType.X,
                                 negate=True)
            pr = sc.tile([P, K * jt], f32, tag="pr")
            z8 = sc.tile([P, K], f32, tag="z8")
            for k in range(K):
                nc.scalar.activation(
                    pr[:, k * jt:(k + 1) * jt], s1[:, k * jt:(k + 1) * jt],
                    mybir.ActivationFunctionType.Exp,
                    bias=negm[:, k:k + 1], scale=1.0,
                    accum_out=z8[:, k:k + 1])
            rz = sc.tile([P, K], f32, tag="rz")
            nc.vector.reciprocal(rz[:], z8[:])
            al = sc.tile([P, K * jt], bf16, tag="al")
            for k in range(K):
                nc.scalar.mul(al[:, k * jt:(k + 1) * jt],
                              pr[:, k * jt:(k + 1) * jt], rz[:, k:k + 1])
            # weighted h, written kf-major so the slot-reduce is unit-stride
            wg = swg.tile([P, KF * jt], bf16, tag="wg")
            nc.vector.tensor_mul(
                wg[:].rearrange("p (k f d) -> p d k f", k=K, f=F_OUT),
                g3[:, :, 0:KF].rearrange("p d (k f) -> p d k f", f=F_OUT),
                al[:].rearrange("p (k d) -> p d k", d=jt)
                    .unsqueeze(-1).to_broadcast([P, jt, K, F_OUT]))
            o = sc.tile([P, KF], f32, tag="o")
            nc.vector.reduce_sum(
                o[:], wg[:].rearrange("p (kf d) -> p kf d", d=jt),
                axis=mybir.AxisListType.X)
            # ELU(o) = max(o,0) + exp(min(o,0)) - 1, on ACT:
            # u = relu(-o) = -min(o,0); e1 = exp(-u); r = relu(o)
            u = sc.tile([P, KF], f32, tag="u")
            nc.scalar.activation(u[:], o[:], mybir.ActivationFunctionType.Relu,
                                 scale=-1.0)
            e1 = sc.tile([P, KF], f32, tag="e1")
            nc.scalar.activation(e1[:], u[:], mybir.ActivationFunctionType.Exp,
                                 scale=-1.0)
            r = sc.tile([P, KF], f32, tag="r")
            nc.scalar.activation(r[:], o[:], mybir.ActivationFunctionType.Relu)
            ot = sc.tile([P, KF], f32, tag="ot")
            nc.vector.scalar_tensor_tensor(
                ot[:], e1[:], -1.0, r[:],
                op0=mybir.AluOpType.add, op1=mybir.AluOpType.add)
            nc.sync.dma_start(out.ap()[t * P:(t + 1) * P, :], ot[:])

    nc.compile()
    return nc


def snake_order(cnt):
    c0, c1, c2, c3 = cnt.T
    d1 = np.where(c0 % 2 == 0, c1, DEG - c1)
    d2 = np.where(d1 % 2 == 0, c2, DEG - c2)
    d3 = np.where(d2 % 2 == 0, c3, DEG - c3)
    key = ((c0 * 17 + d1) * 17 + d2) * 17 + d3
    return np.argsort(key, kind="stable")


def host_plan(nbr):
    """Per-core node ordering, global J table, per-core idx buffers,
    per-core truncation counts and slot masks. Windows are shard-pair
    aligned, so a neighbor's window depends only on its source core.
    Tiles are permuted so the 3 largest-jt tiles run first (they are also
    exempt from truncation): they initialize the rotating gather buffers."""
    nbr = np.asarray(nbr).astype(np.int64)
    src_core = nbr // S
    win = src_core // 2                                     # [N, DEG] in 0..3
    orders0 = []
    cnts = []
    for c in range(N_CORES):
        w = win[c * S:(c + 1) * S]
        cnt = np.stack([(w == q).sum(1) for q in range(NW)], 1)  # [S,NW]
        orders0.append(snake_order(cnt))
        cnts.append(cnt)
    # per-tile window maxima in the unpermuted (sorted) tile order
    Js0 = np.zeros((N_CORES, NT, NW), np.int64)
    for c in range(N_CORES):
        cnt = cnts[c][orders0[c]]
        cs = np.concatenate([cnt, np.zeros((SP - S, NW), np.int64)])
        Js0[c] = cs.reshape(NT, P, NW).max(1)
    J0 = Js0.max(axis=0)
    Jt0 = J0.sum(axis=1)
    # tile permutation: 3 largest-jt tiles first, rest in original order
    top3 = list(np.argsort(-Jt0, kind="stable")[:3])
    tau = top3 + [t for t in range(NT) if t not in top3]
    J = J0[tau]
    Js = Js0[:, tau]
    # device-row node lists (pad rows = -1 where the short tail tile lands)
    pos_of = {t: i for i, t in enumerate(tau)}
    orders = []
    for c in range(N_CORES):
        dev = np.full(SP, -1, np.int64)
        for i, t in enumerate(tau):
            blk = orders0[c][t * P:min((t + 1) * P, S)]
            dev[i * P:i * P + len(blk)] = blk
        orders.append(dev)
    padtile = pos_of[NT - 1]
    ps0 = padtile * P + (S - (NT - 1) * P)    # first pad device row
    inv = np.empty(N, np.int64)
    for c in range(N_CORES):
        valid = orders[c] >= 0
        inv[c * S + orders[c][valid]] = np.nonzero(valid)[0]
    rows = (src_core * SP + inv[nbr]).astype(np.int32)
    dummy = tuple(2 * w * SP + ps0 for w in range(NW))
    percore = []
    for c in range(N_CORES):
        dev = orders[c]
        valid = dev >= 0
        rs = np.zeros((SP, DEG), np.int32)
        cs = np.zeros((SP, NW), np.int64)
        rs[valid] = np.sort(rows[c * S + dev[valid]], axis=1)
        cs[valid] = cnts[c][dev[valid]]
        start = np.concatenate(
            [np.zeros((SP, 1), np.int64), np.cumsum(cs, 1)[:, :-1]], 1)
        percore.append((rs, cs, start))
    # per-core truncation count (>=1 slot so the gather is never empty);
    # first 3 tiles are full so they initialize the rotating buffers
    Jc = np.maximum(Js, 1)                                  # [C, NT, NW]
    Jc[:, 0:3, :] = J[None, 0:3, :]
    idxbufs = []
    nidxbufs = []
    mskbufs = []
    for c in range(N_CORES):
        rs, cs, start = percore[c]
        segs = []
        nidx_vals = []
        msk_vals = []
        for t in range(NT):
            rt = rs[t * P:(t + 1) * P]
            ct = cs[t * P:(t + 1) * P]
            st = start[t * P:(t + 1) * P]
            for w in range(NW):
                jw = int(J[t, w])
                if jw == 0:
                    continue
                jc = int(Jc[c, t, w]) if TRUNC else jw
                jj = np.arange(jw)[None, :]
                take = st[:, w:w + 1] + jj
                valid = jj < ct[:, w:w + 1]
                vals = np.where(
                    valid,
                    np.take_along_axis(rt, np.minimum(take, DEG - 1).astype(
                        np.int64), 1),
                    dummy[w]).astype(np.int64) - BOUNDS[w]
                # slots beyond this core's max count: truncated (negative)
                vals[:, jc:] = -1
                nidx_vals.append(jc * P)
                m = np.zeros(jw, np.float32)
                m[jc:] = NEG_BIG
                msk_vals.append(m)
                # linear order position i = j*128 + p -> 16-partition wrap
                lin = vals.T.reshape(-1)                     # [jw*128]
                seg = lin.reshape(-1, 16).T.astype(np.int16)  # [16, jw*8]
                segs.append(seg)
        buf16 = np.concatenate(segs, axis=1)
        idxbufs.append(np.ascontiguousarray(np.tile(buf16, (8, 1))))
        nidxbufs.append(np.asarray(nidx_vals, np.int32)[None, :])
        mskbufs.append(np.concatenate(msk_vals)[None, :])
    return J, orders, idxbufs, nidxbufs, mskbufs, ps0


def prep_inputs(X, W, a, nbr):
    X = np.asarray(X, dtype=np.float32)
    W = np.asarray(W, dtype=np.float32)
    a = np.asarray(a, dtype=np.float32)
    J, orders, idxbufs, nidxbufs, mskbufs, ps0 = host_plan(nbr)
    wt = np.ascontiguousarray(W.transpose(2, 0, 1).reshape(F_IN, KF))
    wkf = np.ascontiguousarray(W.reshape(KF, F_IN))
    am = np.zeros((KF, 8), np.float32)
    for k in range(K):
        am[k * F_OUT:(k + 1) * F_OUT, k] = a[k, 0, :F_OUT]
        am[k * F_OUT:(k + 1) * F_OUT, 4 + k] = a[k, 0, F_OUT:]
    import ml_dtypes
    pf = np.full((max(SP - S, 1), 8), NEG_BIG, dtype=ml_dtypes.bfloat16)
    in_maps = []
    for c in range(N_CORES):
        dev = orders[c]
        valid = dev >= 0
        xs = np.zeros((SP, F_IN), dtype=np.float32)
        xs[valid] = X[c * S + dev[valid]]
        xst = np.ascontiguousarray(xs.T)
        in_maps.append({"xst": xst, "wt": wt, "wkf": wkf, "am": am,
                        "idxin": idxbufs[c], "nidxin": nidxbufs[c],
                        "mskin": mskbufs[c], "padfill": pf})
    return J, orders, in_maps, ps0


_NC_CACHE = {}


def kernel(X, W, a, nbr):
    from concourse.bass_utils import run_bass_kernel_spmd

    J, orders, in_maps, ps0 = prep_inputs(X, W, a, nbr)
    key = hashlib.sha1(J.tobytes() + bytes([ps0 % 251])).hexdigest()
    if key not in _NC_CACHE:
        _NC_CACHE[key] = build_nc(J, trunc=TRUNC, ps0=ps0)
    nc = _NC_CACHE[key]
    res = run_bass_kernel_spmd(nc, in_maps, core_ids=list(range(N_CORES)))
    out = np.empty((N, KF), dtype=np.float32)
    for c in range(N_CORES):
        dev = orders[c]
        valid = dev >= 0
        out[c * S + dev[valid]] = res.results[c]["out"][valid]
    return out


# revision 8
# speedup vs baseline: 1.1279x; 1.1279x over previous
"""GAT kernel v2 for Trainium2, 8-core SPMD.

Changes vs baseline:
  - Snake (boustrophedon) node binning: gather padding 1.52x -> 1.35x.
  - Optional per-core trailing truncation of gather calls via reg_load'd
    num_idxs_reg (TRUNC): effective padding -> per-core ~1.23x.
  - Stage A: host-pretransposed X (no per-tile PE transpose); e_src/e_dst
    computed by PE in the same matmul as h (fused [wt | wa] rhs, wa built
    on device from host-permuted W/a layouts); he rows written 272B.
  - Stage C: exp on ACT engine with per-partition bias=-max and accum_out
    z (removes two DVE passes); reduce_max emits negated max directly.
"""
import sys

if "/opt/trn_rl_repo" not in sys.path:
    sys.path.insert(0, "/opt/trn_rl_repo")

import hashlib
import numpy as np

N, DEG, K, F_IN, F_OUT = 100000, 16, 4, 128, 32
KF = K * F_OUT            # 128
N_CORES = 8
S = N // N_CORES          # 12500
P = 128
NT = (S + P - 1) // P     # 98
SP = NT * P               # 12544
NTAB = N_CORES * SP       # 100352
EL = 256                  # bf16 elements per table row (512B)
EOFF = 128                # h at [0,128); e_dst hi bf16 [128,132); lo [132,136)
HEW = 136                 # written row width (h + e hi/lo)
BOUNDS = (0, 2 * SP, 4 * SP, 6 * SP, NTAB)   # shard-pair aligned (25088)
NW = 4
DUMMY = (S, 2 * SP + S, 4 * SP + S, 6 * SP + S)
NEG_SLOPE = 0.01
NEG_BIG = -1.0e30
TRUNC = False
SINGLE_PACKET = False


def build_nc(J, n_cores=N_CORES, nt=NT, trunc=TRUNC, ps0=S):
    """J: [nt, NW] int array of per-tile window slot counts (uniform across
    cores). ps0: first pad row in the (tile-permuted) device row order.
    Builds and compiles the SPMD program."""
    from contextlib import ExitStack

    import concourse.bass as bass
    import concourse.tile as tile
    from concourse import bacc, mybir

    f32 = mybir.dt.float32
    bf16 = mybir.dt.bfloat16
    i16 = mybir.dt.int16
    i32 = mybir.dt.int32
    sp = nt * P
    Jt = J.sum(axis=1)            # slots per tile
    JTM = int(Jt.max())
    CTOT = int(J.sum()) * 8       # idxbuf columns (16-wrapped)
    NCALL = int((J > 0).sum())

    nc = bacc.Bacc("TRN2", target_bir_lowering=False, debug=False,
                   num_devices=n_cores, num_swdge_queues=4)

    CTJ = int(Jt.sum())
    xst = nc.dram_tensor("xst", [F_IN, sp], f32, kind="ExternalInput")
    wt = nc.dram_tensor("wt", [F_IN, KF], f32, kind="ExternalInput")
    wkf = nc.dram_tensor("wkf", [KF, F_IN], f32, kind="ExternalInput")
    am = nc.dram_tensor("am", [KF, 8], f32, kind="ExternalInput")
    idxin = nc.dram_tensor("idxin", [P, CTOT], i16, kind="ExternalInput")
    nidxin = nc.dram_tensor("nidxin", [1, NCALL], i32, kind="ExternalInput")
    mskin = nc.dram_tensor("mskin", [1, CTJ], f32, kind="ExternalInput")
    padfill = nc.dram_tensor("padfill", [sp - S if sp > S else 1, 8], bf16,
                             kind="ExternalInput")
    out = nc.dram_tensor("out", [sp, KF], f32, kind="ExternalOutput")

    he_shard = nc.dram_tensor("he_shard", [sp, EL], bf16, kind="Internal")
    he_full = nc.dram_tensor("he_full", [NTAB, EL], bf16, kind="Internal",
                             addr_space="Shared")

    with tile.TileContext(nc) as tc, ExitStack() as ctx:
        consts = ctx.enter_context(tc.tile_pool(name="consts", bufs=1))
        sa = ctx.enter_context(tc.tile_pool(name="sa", bufs=4))
        sa_ps = ctx.enter_context(tc.tile_pool(name="sa_ps", bufs=4, space="PSUM"))
        sc = ctx.enter_context(tc.tile_pool(name="sc", bufs=4))
        scg = ctx.enter_context(tc.tile_pool(name="scg", bufs=6))
        swg = ctx.enter_context(tc.tile_pool(name="swg", bufs=4))

        wkf_sb = consts.tile([KF, F_IN], f32)
        nc.sync.dma_start(wkf_sb[:], wkf.ap())
        am_sb = consts.tile([KF, 8], f32)
        nc.sync.dma_start(am_sb[:], am.ap())
        rhs_sb = consts.tile([F_IN, KF + 8], f32)
        nc.sync.dma_start(rhs_sb[:, 0:KF], wt.ap())
        wa_ps = sa_ps.tile([F_IN, 8], f32, tag="wa")
        nc.tensor.matmul(wa_ps[:], lhsT=wkf_sb[:], rhs=am_sb[:],
                         start=True, stop=True)
        nc.vector.tensor_copy(rhs_sb[:, KF:KF + 8], wa_ps[:])

        es_sb = consts.tile([P, nt * K], f32)
        idx_sb = consts.tile([P, CTOT], i16)
        nc.sync.dma_start(idx_sb[:], idxin.ap())
        nidx_sb = consts.tile([1, NCALL], i32)
        nc.sync.dma_start(nidx_sb[:], nidxin.ap())
        msk_sb = None
        if trunc:
            # per-slot mask (0 valid / -1e30 truncated), replicated to all
            # partitions by a partition-stride-0 DMA read
            msk_sb = consts.tile([P, CTJ], f32)
            nc.sync.dma_start(msk_sb[:], bass.AP(mskin, 0, [[0, P], [1, CTJ]]))

        # ---- Stage A ----
        # 2 node-tiles per DMA instruction (in and out): halves the SP
        # queue's per-DMA dispatch/sem chain without touching stage C pools
        GA = 2
        for t0 in range(0, nt, GA):
            gg = min(GA, nt - t0)
            xt4 = sa.tile([F_IN, GA * P], f32, tag="x")
            nc.sync.dma_start(xt4[:, 0:gg * P],
                              xst.ap()[:, t0 * P:(t0 + gg) * P])
            he4 = sa.tile([P, GA * HEW], bf16, tag="he")
            for g in range(gg):
                t = t0 + g
                xt_sb = xt4[:, g * P:(g + 1) * P]
                he8_ps = sa_ps.tile([P, KF + 8], f32, tag="he8")
                nc.tensor.matmul(he8_ps[:], lhsT=xt_sb, rhs=rhs_sb[:],
                                 start=True, stop=True)
                he_t = he4[:, g * HEW:(g + 1) * HEW]
                nc.scalar.copy(he_t[:, 0:KF], he8_ps[:, 0:KF])  # f32 -> bf16
                nc.vector.tensor_copy(es_sb[:, t * K:(t + 1) * K],
                                      he8_ps[:, KF:KF + 4])
                # e_dst as hi+lo bf16 pair (~16-bit mantissa total)
                nc.vector.tensor_copy(he_t[:, EOFF:EOFF + 4],
                                      he8_ps[:, KF + 4:KF + 8])
                ehi32 = sa.tile([P, K], f32, tag="ehi32")
                nc.vector.tensor_copy(ehi32[:], he_t[:, EOFF:EOFF + 4])
                elo = sa.tile([P, K], f32, tag="elo")
                nc.vector.tensor_sub(elo[:], he8_ps[:, KF + 4:KF + 8],
                                     ehi32[:])
                nc.vector.tensor_copy(he_t[:, EOFF + 4:EOFF + 8], elo[:])
            dst = bass.AP(he_shard, t0 * P * EL,
                          [[EL, P], [P * EL, gg], [1, HEW]])
            nc.sync.dma_start(
                dst, he4[:, 0:gg * HEW].rearrange("p (g c) -> p g c", c=HEW))
        npad = sp - S
        if npad > 0:
            pf = consts.tile([npad, 8], bf16)
            nc.sync.dma_start(pf[:], padfill.ap())
            nc.sync.dma_start(
                he_shard.ap()[ps0:ps0 + npad, EOFF:EOFF + 8], pf[:])

        # ---- Stage B ----
        nc.gpsimd.collective_compute(
            "AllGather", mybir.AluOpType.bypass,
            replica_groups=[list(range(n_cores))],
            ins=[he_shard.ap()], outs=[he_full.ap()])

        # ---- Stage C ----
        # Tiles are host-ordered so the 3 largest-jt tiles run first and
        # untruncated: they fully initialize the 3 rotating gather buffers,
        # so later (truncated) tiles only ever read slots some predecessor
        # wrote (no NaN from uninitialized SBUF, no warm-up memsets needed).
        regs = [ctx.enter_context(nc.gpsimd.register(f"nreg{i}"))
                for i in range(4)]

        coff = 0
        moff = 0
        ncall = 0
        for t in range(nt):
            jt = int(Jt[t])
            g = scg.tile([P, JTM * EL], bf16, tag="g")
            g3 = g[:, 0:jt * EL].rearrange("p (d w) -> p d w", w=EL)
            b = 0
            for w in range(NW):
                jw = int(J[t, w])
                if jw == 0:
                    continue
                nidx = jw * P
                if trunc:
                    reg = regs[ncall % 4]
                    nc.gpsimd.reg_load(reg, nidx_sb[0:1, ncall:ncall + 1])
                    nreg = reg
                else:
                    nreg = nidx
                nc.gpsimd.dma_gather(
                    out_ap=g3[:, b:b + jw, :],
                    in_ap=he_full.ap()[BOUNDS[w]:BOUNDS[w + 1], :],
                    idxs_ap=idx_sb[:, coff:coff + jw * 8],
                    num_idxs=nidx, num_idxs_reg=nreg, elem_size=EL,
                    single_packet=SINGLE_PACKET, queue_num=(w + t) % 4)
                ncall += 1
                b += jw
                coff += jw * 8
            # scores (k-major over slots): e_dst = hi + lo, then + e_src
            ehi = g3[:, :, EOFF:EOFF + 4].rearrange("p d k -> p k d")
            elo_v = g3[:, :, EOFF + 4:EOFF + 8].rearrange("p d k -> p k d")
            ed = sc.tile([P, K * jt], f32, tag="ed")
            edv = ed[:].rearrange("p (k d) -> p k d", d=jt)
            nc.vector.tensor_add(edv, ehi, elo_v)
            esrc = es_sb[:, t * K:(t + 1) * K].unsqueeze(-1).to_broadcast(
                [P, K, jt])
            s0 = sc.tile([P, K * jt], f32, tag="s0")
            s0v = s0[:].rearrange("p (k d) -> p k d", d=jt)
            nc.vector.tensor_add(s0v, edv, esrc)
            if trunc:
                mskv = msk_sb[:, moff:moff + jt].unsqueeze(1).to_broadcast(
                    [P, K, jt])
                nc.vector.tensor_add(s0v, s0v, mskv)
                moff += jt
            s1 = sc.tile([P, K * jt], f32, tag="s1")
            nc.vector.scalar_tensor_tensor(
                s1[:], s0[:], NEG_SLOPE, s0[:],
                op0=mybir.AluOpType.mult, op1=mybir.AluOpType.max)
            s1v = s1[:].rearrange("p (k d) -> p k d", d=jt)
            negm = sc.tile([P, K], f32, tag="negm")
            nc.vector.reduce_max(negm[:], s1v, axis=mybir.AxisListType.X,
                                 negate=True)
            pr = sc.tile([P, K * jt], f32, tag="pr")
            z8 = sc.tile([P, K], f32, tag="z8")
            for k in range(K):
                nc.scalar.activation(
                    pr[:, k * jt:(k + 1) * jt], s1[:, k * jt:(k + 1) * jt],
                    mybir.ActivationFunctionType.Exp,
                    bias=negm[:, k:k + 1], scale=1.0,
                    accum_out=z8[:, k:k + 1])
            rz = sc.tile([P, K], f32, tag="rz")
            nc.vector.reciprocal(rz[:], z8[:])
            al = sc.tile([P, K * jt], bf16, tag="al")
            for k in range(K):
                nc.scalar.mul(al[:, k * jt:(k + 1) * jt],
                              pr[:, k * jt:(k + 1) * jt], rz[:, k:k + 1])
            # weighted h, written kf-major so the slot-reduce is unit-stride
            wg = swg.tile([P, KF * jt], bf16, tag="wg")
            nc.vector.tensor_mul(
                wg[:].rearrange("p (k f d) -> p d k f", k=K, f=F_OUT),
                g3[:, :, 0:KF].rearrange("p d (k f) -> p d k f", f=F_OUT),
                al[:].rearrange("p (k d) -> p d k", d=jt)
                    .unsqueeze(-1).to_broadcast([P, jt, K, F_OUT]))
            o = sc.tile([P, KF], f32, tag="o")
            nc.vector.reduce_sum(
                o[:], wg[:].rearrange("p (kf d) -> p kf d", d=jt),
                axis=mybir.AxisListType.X)
            # ELU(o) = max(o,0) + exp(min(o,0)) - 1, on ACT:
            # u = relu(-o) = -min(o,0); e1 = exp(-u); r = relu(o)
            u = sc.tile([P, KF], f32, tag="u")
            nc.scalar.activation(u[:], o[:], mybir.ActivationFunctionType.Relu,
                                 scale=-1.0)
            e1 = sc.tile([P, KF], f32, tag="e1")
            nc.scalar.activation(e1[:], u[:], mybir.ActivationFunctionType.Exp,
                                 scale=-1.0)
            r = sc.tile([P, KF], f32, tag="r")
            nc.scalar.activation(r[:], o[:], mybir.ActivationFunctionType.Relu)
            ot = sc.tile([P, KF], f32, tag="ot")
            nc.vector.scalar_tensor_tensor(
                ot[:], e1[:], -1.0, r[:],
                op0=mybir.AluOpType.add, op1=mybir.AluOpType.add)
            nc.sync.dma_start(out.ap()[t * P:(t + 1) * P, :], ot[:])

    nc.compile()
    return nc


def snake_order(cnt):
    c0, c1, c2, c3 = cnt.T
    d1 = np.where(c0 % 2 == 0, c1, DEG - c1)
    d2 = np.where(d1 % 2 == 0, c2, DEG - c2)
    d3 = np.where(d2 % 2 == 0, c3, DEG - c3)
    key = ((c0 * 17 + d1) * 17 + d2) * 17 + d3
    return np.argsort(key, kind="stable")


def host_plan(nbr):
    """Per-core node ordering, global J table, per-core idx buffers,
    per-core truncation counts and slot masks. Windows are shard-pair
    aligned, so a neighbor's window depends only on its source core.
    Tiles are permuted so the 3 largest-jt tiles run first (they are also
    exempt from truncation): they initialize the rotating gather buffers."""
    nbr = np.asarray(nbr).astype(np.int64)
    src_core = nbr // S
    win = src_core // 2                                     # [N, DEG] in 0..3
    orders0 = []
    cnts = []
    for c in range(N_CORES):
        w = win[c * S:(c + 1) * S]
        cnt = np.stack([(w == q).sum(1) for q in range(NW)], 1)  # [S,NW]
        orders0.append(snake_order(cnt))
        cnts.append(cnt)
    # per-tile window maxima in the unpermuted (sorted) tile order
    Js0 = np.zeros((N_CORES, NT, NW), np.int64)
    for c in range(N_CORES):
        cnt = cnts[c][orders0[c]]
        cs = np.concatenate([cnt, np.zeros((SP - S, NW), np.int64)])
        Js0[c] = cs.reshape(NT, P, NW).max(1)
    J0 = Js0.max(axis=0)
    Jt0 = J0.sum(axis=1)
    # tile permutation: 3 largest-jt tiles first, rest in original order
    top3 = list(np.argsort(-Jt0, kind="stable")[:3])
    tau = top3 + [t for t in range(NT) if t not in top3]
    J = J0[tau]
    Js = Js0[:, tau]
    # device-row node lists (pad rows = -1 where the short tail tile lands)
    pos_of = {t: i for i, t in enumerate(tau)}
    orders = []
    for c in range(N_CORES):
        dev = np.full(SP, -1, np.int64)
        for i, t in enumerate(tau):
            blk = orders0[c][t * P:min((t + 1) * P, S)]
            dev[i * P:i * P + len(blk)] = blk
        orders.append(dev)
    padtile = pos_of[NT - 1]
    ps0 = padtile * P + (S - (NT - 1) * P)    # first pad device row
    inv = np.empty(N, np.int64)
    for c in range(N_CORES):
        valid = orders[c] >= 0
        inv[c * S + orders[c][valid]] = np.nonzero(valid)[0]
    rows = (src_core * SP + inv[nbr]).astype(np.int32)
    dummy = tuple(2 * w * SP + ps0 for w in range(NW))
    percore = []
    for c in range(N_CORES):
        dev = orders[c]
        valid = dev >= 0
        rs = np.zeros((SP, DEG), np.int32)
        cs = np.zeros((SP, NW), np.int64)
        rs[valid] = np.sort(rows[c * S + dev[valid]], axis=1)
        cs[valid] = cnts[c][dev[valid]]
        start = np.concatenate(
            [np.zeros((SP, 1), np.int64), np.cumsum(cs, 1)[:, :-1]], 1)
        percore.append((rs, cs, start))
    # per-core truncation count (>=1 slot so the gather is never empty);
    # first 3 tiles are full so they initialize the rotating buffers
    Jc = np.maximum(Js, 1)                                  # [C, NT, NW]
    Jc[:, 0:3, :] = J[None, 0:3, :]
    idxbufs = []
    nidxbufs = []
    mskbufs = []
    for c in range(N_CORES):
        rs, cs, start = percore[c]
        segs = []
        nidx_vals = []
        msk_vals = []
        for t in range(NT):
            rt = rs[t * P:(t + 1) * P]
            ct = cs[t * P:(t + 1) * P]
            st = start[t * P:(t + 1) * P]
            for w in range(NW):
                jw = int(J[t, w])
                if jw == 0:
                    continue
                jc = int(Jc[c, t, w]) if TRUNC else jw
                jj = np.arange(jw)[None, :]
                take = st[:, w:w + 1] + jj
                valid = jj < ct[:, w:w + 1]
                vals = np.where(
                    valid,
                    np.take_along_axis(rt, np.minimum(take, DEG - 1).astype(
                        np.int64), 1),
                    dummy[w]).astype(np.int64) - BOUNDS[w]
                # slots beyond this core's max count: truncated (negative)
                vals[:, jc:] = -1
                nidx_vals.append(jc * P)
                m = np.zeros(jw, np.float32)
                m[jc:] = NEG_BIG
                msk_vals.append(m)
                # linear order position i = j*128 + p -> 16-partition wrap
                lin = vals.T.reshape(-1)                     # [jw*128]
                seg = lin.reshape(-1, 16).T.astype(np.int16)  # [16, jw*8]
                segs.append(seg)
        buf16 = np.concatenate(segs, axis=1)
        idxbufs.append(np.ascontiguousarray(np.tile(buf16, (8, 1))))
        nidxbufs.append(np.asarray(nidx_vals, np.int32)[None, :])
        mskbufs.append(np.concatenate(msk_vals)[None, :])
    return J, orders, idxbufs, nidxbufs, mskbufs, ps0


def prep_inputs(X, W, a, nbr):
    X = np.asarray(X, dtype=np.float32)
    W = np.asarray(W, dtype=np.float32)
    a = np.asarray(a, dtype=np.float32)
    J, orders, idxbufs, nidxbufs, mskbufs, ps0 = host_plan(nbr)
    wt = np.ascontiguousarray(W.transpose(2, 0, 1).reshape(F_IN, KF))
    wkf = np.ascontiguousarray(W.reshape(KF, F_IN))
    am = np.zeros((KF, 8), np.float32)
    for k in range(K):
        am[k * F_OUT:(k + 1) * F_OUT, k] = a[k, 0, :F_OUT]
        am[k * F_OUT:(k + 1) * F_OUT, 4 + k] = a[k, 0, F_OUT:]
    import ml_dtypes
    pf = np.full((max(SP - S, 1), 8), NEG_BIG, dtype=ml_dtypes.bfloat16)
    in_maps = []
    for c in range(N_CORES):
        dev = orders[c]
        valid = dev >= 0
        xs = np.zeros((SP, F_IN), dtype=np.float32)
        xs[valid] = X[c * S + dev[valid]]
        xst = np.ascontiguousarray(xs.T)
        in_maps.append({"xst": xst, "wt": wt, "wkf": wkf, "am": am,
                        "idxin": idxbufs[c], "nidxin": nidxbufs[c],
                        "mskin": mskbufs[c], "padfill": pf})
    return J, orders, in_maps, ps0


_NC_CACHE = {}


def kernel(X, W, a, nbr):
    from concourse.bass_utils import run_bass_kernel_spmd

    J, orders, in_maps, ps0 = prep_inputs(X, W, a, nbr)
    key = hashlib.sha1(J.tobytes() + bytes([ps0 % 251])).hexdigest()
    if key not in _NC_CACHE:
        _NC_CACHE[key] = build_nc(J, trunc=TRUNC, ps0=ps0)
    nc = _NC_CACHE[key]
    res = run_bass_kernel_spmd(nc, in_maps, core_ids=list(range(N_CORES)))
    out = np.empty((N, KF), dtype=np.float32)
    for c in range(N_CORES):
        dev = orders[c]
        valid = dev >= 0
        out[c * S + dev[valid]] = res.results[c]["out"][valid]
    return out
